# revision 11
# baseline (speedup 1.0000x reference)
"""Trainium2 Bass kernel for nn_CILRSModel (moe_routing).

Strategy:
  - Host-side MoE routing: rows are bucketed by `command` (6 branches) and
    distributed evenly over 8 cores. Each core gets a fixed [6 x CAP] row
    layout so the SPMD kernel statically knows which branch weights apply
    to which batch tile (no on-device routing control flow).
  - Host-side transpose: the embedding (+ speed scalar) is shipped
    feature-major ([513, rows_per_core]) so every matmul operand already
    has the contraction dim on SBUF partitions - zero on-device transposes.
  - On device, everything is feature-major: x [640, N] tiles flow through
    speed-MLP, speed head and the (single, routed) branch head on the PE;
    PSUM is evicted with fused bias+ReLU (tensor_scalar / activation)
    split across the Vector and Scalar engines.
  - Outputs come back feature-major as out4 = [4, rows] (3 control rows +
    1 speed row) and are scattered back to the original row order on host.
"""

import os
import sys

import numpy as np

_TRN_REPO = "/opt/trn_rl_repo"
if _TRN_REPO not in sys.path:
    sys.path.insert(0, _TRN_REPO)

# Problem constants (hardcoded per harness contract)
B = 65536
D_EMB = 512
D_LAT = 128
H = 256
NBRANCH = 6
D_IN = D_EMB + D_LAT  # 640
NCORES = 8
CAP = 1536            # per-core per-branch row capacity (actual max ~1389)
BPC = NBRANCH * CAP   # 9216 rows per core
NT = 512              # batch-tile size (matmul free dim)
NTILES = BPC // NT    # 18
TPB = CAP // NT       # tiles per branch slot = 3

_cache = {}


# --------------------------------------------------------------------------
# Device kernel
# --------------------------------------------------------------------------

def _build_nc():
    if "nc" in _cache:
        return _cache["nc"]

    import concourse.mybir as mybir
    import concourse.tile as tile
    from concourse import bacc
    from concourse.bass import ts

    f32 = mybir.dt.float32
    f16 = mybir.dt.float16   # matmul operand dtype (non-f32 -> LDW split, 2x PE)
    AF = mybir.ActivationFunctionType
    ALU = mybir.AluOpType

    nc = bacc.Bacc("TRN2", target_bir_lowering=False, debug=False,
                   num_devices=NCORES)

    def din(name, shape, dt=f16):
        return nc.dram_tensor(name, list(shape), dt, kind="ExternalInput")[:]

    xt = din("xt", [D_EMB + 1, BPC])           # rows 0..511 emb.T, row 512 speed
    wsi1 = din("wsi1", [1, H])
    bsi1 = din("bsi1", [128, 2], f32)
    wsi2 = din("wsi2", [128, 2, D_LAT])
    bsi2 = din("bsi2", [128, 1], f32)
    wso1 = din("wso1", [128, 5, H])
    bso1 = din("bso1", [128, 2], f32)
    wso2 = din("wso2", [128, 2, H])
    bso2 = din("bso2", [128, 2], f32)
    wso3 = din("wso3", [128, 2, 1])
    bso3 = din("bso3", [1, 1], f32)
    wb1 = din("wb1", [128, 5, NBRANCH, H])
    bb1 = din("bb1", [128, 2, NBRANCH], f32)
    wb2 = din("wb2", [128, 2, NBRANCH, H])
    bb2 = din("bb2", [128, 2, NBRANCH], f32)
    wb3 = din("wb3", [128, 2, NBRANCH, 3])
    bb3 = din("bb3", [3, NBRANCH], f32)
    out4 = nc.dram_tensor("out4", [4, BPC], f32, kind="ExternalOutput")[:]

    with tile.TileContext(nc) as tc:
        with (
            tc.tile_pool(name="wpool", bufs=1) as wpool,
            tc.tile_pool(name="xpool", bufs=3) as xpool,
            tc.tile_pool(name="hpool", bufs=2) as hpool,
            tc.tile_pool(name="opool", bufs=1) as opool,
            tc.tile_pool(name="pmm", bufs=3, space="PSUM") as pmm,
            tc.tile_pool(name="pm1", bufs=2, space="PSUM") as pm1,
        ):
            def loadw(ap, shape, tag, dt=f16):
                t = wpool.tile(list(shape), dt, tag=tag)
                nc.sync.dma_start(out=t[:], in_=ap)
                return t

            wsi1_s = loadw(wsi1, [1, H], "wsi1")
            bsi1_s = loadw(bsi1, [128, 2], "bsi1", f32)
            wsi2_s = loadw(wsi2, [128, 2, D_LAT], "wsi2")
            bsi2_s = loadw(bsi2, [128, 1], "bsi2", f32)
            wso1_s = loadw(wso1, [128, 5, H], "wso1")
            bso1_s = loadw(bso1, [128, 2], "bso1", f32)
            wso2_s = loadw(wso2, [128, 2, H], "wso2")
            bso2_s = loadw(bso2, [128, 2], "bso2", f32)
            wso3_s = loadw(wso3, [128, 2, 1], "wso3")
            bso3_s = loadw(bso3, [1, 1], "bso3", f32)
            wb1_s = loadw(wb1, [128, 5, NBRANCH, H], "wb1")
            bb1_s = loadw(bb1, [128, 2, NBRANCH], "bb1", f32)
            wb2_s = loadw(wb2, [128, 2, NBRANCH, H], "wb2")
            bb2_s = loadw(bb2, [128, 2, NBRANCH], "bb2", f32)
            wb3_s = loadw(wb3, [128, 2, NBRANCH, 3], "wb3")
            bb3_s = loadw(bb3, [3, NBRANCH], "bb3", f32)

            ctl_s = opool.tile([3, BPC], f32, tag="octl")
            spd_s = opool.tile([1, BPC], f32, tag="ospd")

            xt_emb = xt[0:D_EMB, :].rearrange("(o p) b -> p o b", p=128)

            def evict_relu(dst, src, bias_ap, on_act):
                if on_act:
                    nc.scalar.activation(dst, src, AF.Relu, bias=bias_ap)
                else:
                    nc.vector.tensor_scalar(dst, src, bias_ap, 0.0, ALU.add, ALU.max)

            for t in range(NTILES):
                k = t // TPB
                cols = ts(t, NT)

                x_s = xpool.tile([128, 5, NT], f16, tag="x")
                nc.sync.dma_start(out=x_s[:, 0:4, :], in_=xt_emb[:, :, cols])
                nc.sync.dma_start(out=x_s[0:1, 4, :], in_=xt[D_EMB:D_EMB + 1, cols])

                # ---- speed-input MLP, layer 1 (K=1 matmuls on the speed row)
                p_h = pmm.tile([128, 2, NT], f32, tag="pmm")
                nc.tensor.matmul(p_h[:, 0, :], wsi1_s[0:1, 0:128], x_s[0:1, 4, :],
                                 start=True, stop=True)
                nc.tensor.matmul(p_h[:, 1, :], wsi1_s[0:1, 128:256], x_s[0:1, 4, :],
                                 start=True, stop=True)
                hsp = hpool.tile([128, 2, NT], f16, tag="hsp")
                evict_relu(hsp[:, 0, :], p_h[:, 0, :], bsi1_s[:, 0:1], True)
                evict_relu(hsp[:, 1, :], p_h[:, 1, :], bsi1_s[:, 1:2], False)

                # ---- speed head layer 1, emb subtiles only (o=0..3), keeps PE
                # busy while the speed latent is computed
                p1 = pmm.tile([128, 2, NT], f32, tag="pmm")
                for j in range(2):
                    for o in range(4):
                        nc.tensor.matmul(p1[:, j, :], wso1_s[:, o, ts(j, 128)],
                                         x_s[:, o, :], start=(o == 0), stop=False)

                # ---- speed-input MLP, layer 2 -> speed latent into x_s[:,4,:]
                p_sp = pm1.tile([128, NT], f32, tag="pm1")
                nc.tensor.matmul(p_sp[:], wsi2_s[:, 0, :], hsp[:, 0, :],
                                 start=True, stop=False)
                nc.tensor.matmul(p_sp[:], wsi2_s[:, 1, :], hsp[:, 1, :],
                                 start=False, stop=True)

                # ---- branch layer 1, emb subtiles only
                q1 = pmm.tile([128, 2, NT], f32, tag="pmm")
                for j in range(2):
                    for o in range(4):
                        nc.tensor.matmul(q1[:, j, :], wb1_s[:, o, k, ts(j, 128)],
                                         x_s[:, o, :], start=(o == 0), stop=False)

                # speed latent eviction (bias add, no relu)
                nc.vector.tensor_scalar(x_s[:, 4, :], p_sp[:], bsi2_s[:, 0:1],
                                        None, ALU.add)

                # ---- finish layer-1 accumulations with the latent subtile o=4
                for j in range(2):
                    nc.tensor.matmul(p1[:, j, :], wso1_s[:, 4, ts(j, 128)],
                                     x_s[:, 4, :], start=False, stop=True)
                for j in range(2):
                    nc.tensor.matmul(q1[:, j, :], wb1_s[:, 4, k, ts(j, 128)],
                                     x_s[:, 4, :], start=False, stop=True)

                h1 = hpool.tile([128, 2, NT], f16, tag="h1")
                evict_relu(h1[:, 0, :], p1[:, 0, :], bso1_s[:, 0:1], True)
                evict_relu(h1[:, 1, :], p1[:, 1, :], bso1_s[:, 1:2], False)

                # ---- speed head layer 2
                p2 = pmm.tile([128, 2, NT], f32, tag="pmm")
                for j in range(2):
                    for o in range(2):
                        nc.tensor.matmul(p2[:, j, :], wso2_s[:, o, ts(j, 128)],
                                         h1[:, o, :], start=(o == 0), stop=(o == 1))

                g1 = hpool.tile([128, 2, NT], f16, tag="g1")
                evict_relu(g1[:, 0, :], q1[:, 0, :], bb1_s[:, 0, k:k + 1], True)
                evict_relu(g1[:, 1, :], q1[:, 1, :], bb1_s[:, 1, k:k + 1], False)

                # ---- branch layer 2
                q2 = pmm.tile([128, 2, NT], f32, tag="pmm")
                for j in range(2):
                    for o in range(2):
                        nc.tensor.matmul(q2[:, j, :], wb2_s[:, o, k, ts(j, 128)],
                                         g1[:, o, :], start=(o == 0), stop=(o == 1))

                h2 = hpool.tile([128, 2, NT], f16, tag="h2")
                evict_relu(h2[:, 0, :], p2[:, 0, :], bso2_s[:, 0:1], True)
                evict_relu(h2[:, 1, :], p2[:, 1, :], bso2_s[:, 1:2], False)

                # ---- speed head output (M=1)
                p_o = pm1.tile([1, NT], f32, tag="pm1")
                nc.tensor.matmul(p_o[:], wso3_s[:, 0, :], h2[:, 0, :],
                                 start=True, stop=False)
                nc.tensor.matmul(p_o[:], wso3_s[:, 1, :], h2[:, 1, :],
                                 start=False, stop=True)

                g2 = hpool.tile([128, 2, NT], f16, tag="g2")
                evict_relu(g2[:, 0, :], q2[:, 0, :], bb2_s[:, 0, k:k + 1], True)
                evict_relu(g2[:, 1, :], q2[:, 1, :], bb2_s[:, 1, k:k + 1], False)

                nc.vector.tensor_scalar(spd_s[:, cols], p_o[:], bso3_s[0:1, 0:1],
                                        None, ALU.add)

                # ---- branch output (M=3) + sigmoid
                p_c = pm1.tile([3, NT], f32, tag="pm1")
                nc.tensor.matmul(p_c[:], wb3_s[:, 0, k, :], g2[:, 0, :],
                                 start=True, stop=False)
                nc.tensor.matmul(p_c[:], wb3_s[:, 1, k, :], g2[:, 1, :],
                                 start=False, stop=True)
                nc.scalar.activation(ctl_s[:, cols], p_c[:], AF.Sigmoid,
                                     bias=bb3_s[:, k:k + 1])

            nc.sync.dma_start(out=out4[0:3, :], in_=ctl_s[:])
            nc.sync.dma_start(out=out4[3:4, :], in_=spd_s[:])

    nc.compile()
    _cache["nc"] = nc
    return nc


# --------------------------------------------------------------------------
# Host-side routing / layout
# --------------------------------------------------------------------------

def _fm(w, dtype):
    """[K, ...] -> [128, K//128, ...] with contraction index f = o*128 + p."""
    ko = w.shape[0] // 128
    perm = (1, 0) + tuple(range(2, w.ndim + 1))
    return np.ascontiguousarray(
        w.reshape(ko, 128, *w.shape[1:]).transpose(*perm), dtype=dtype)


def _prep_weights(i):
    f32 = np.float32
    f16 = np.float16

    def a(x):
        return np.asarray(x, dtype=f32)

    wb1 = a(i["Wb1"]).transpose(1, 0, 2)   # [640, 6, 256]
    wb2 = a(i["Wb2"]).transpose(1, 0, 2)   # [256, 6, 256]
    wb3 = a(i["Wb3"]).transpose(1, 0, 2)   # [256, 6, 3]
    bb1 = a(i["bb1"]).T                    # [256, 6]
    bb2 = a(i["bb2"]).T
    return {
        "wsi1": np.ascontiguousarray(a(i["Wsi1"]), dtype=f16),   # [1, 256]
        "bsi1": _fm(a(i["bsi1"]), f32),                          # [128, 2]
        "wsi2": _fm(a(i["Wsi2"]), f16),                          # [128, 2, 128]
        "bsi2": a(i["bsi2"]).reshape(128, 1).copy(),             # [128, 1]
        "wso1": _fm(a(i["Wso1"]), f16),                          # [128, 5, 256]
        "bso1": _fm(a(i["bso1"]), f32),                          # [128, 2]
        "wso2": _fm(a(i["Wso2"]), f16),                          # [128, 2, 256]
        "bso2": _fm(a(i["bso2"]), f32),                          # [128, 2]
        "wso3": _fm(a(i["Wso3"]), f16),                          # [128, 2, 1]
        "bso3": a(i["bso3"]).reshape(1, 1).copy(),               # [1, 1]
        "wb1": _fm(wb1, f16),                                    # [128, 5, 6, 256]
        "bb1": _fm(bb1, f32),                                    # [128, 2, 6]
        "wb2": _fm(wb2, f16),                                    # [128, 2, 6, 256]
        "bb2": _fm(bb2, f32),                                    # [128, 2, 6]
        "wb3": _fm(wb3, f16),                                    # [128, 2, 6, 3]
        "bb3": np.ascontiguousarray(a(i["bb3"]).T),              # [3, 6]
    }


def _route(cmd):
    """Assign rows to (core, slot-position); slot k of every core holds only
    branch-k rows. Returns idx [NCORES, BPC], valid [NCORES, BPC], spill."""
    idx = np.zeros((NCORES, BPC), dtype=np.int64)
    valid = np.zeros((NCORES, BPC), dtype=bool)
    spill = []
    for k in range(NBRANCH):
        rows = np.flatnonzero(cmd == k)
        for c, part in enumerate(np.array_split(rows, NCORES)):
            if len(part) > CAP:
                spill.append(part[CAP:])
                part = part[:CAP]
            idx[c, k * CAP:k * CAP + len(part)] = part
            valid[c, k * CAP:k * CAP + len(part)] = True
    spill = np.concatenate(spill) if spill else np.zeros(0, dtype=np.int64)
    return idx, valid, spill


def _np_reference(i, rows):
    """Exact reference math in numpy for a subset of rows (spill fallback)."""
    f32 = np.float32
    E = np.asarray(i["embedding"], f32)[rows]
    S = np.asarray(i["speed"], f32)[rows]
    cmd = np.asarray(i["command"])[rows].astype(np.int64) - 1
    sp = np.maximum(S @ np.asarray(i["Wsi1"], f32) + np.asarray(i["bsi1"], f32), 0)
    sp = sp @ np.asarray(i["Wsi2"], f32) + np.asarray(i["bsi2"], f32)
    emb = np.concatenate([E, sp], axis=1)
    hs = np.maximum(emb @ np.asarray(i["Wso1"], f32) + np.asarray(i["bso1"], f32), 0)
    hs = np.maximum(hs @ np.asarray(i["Wso2"], f32) + np.asarray(i["bso2"], f32), 0)
    spd = hs @ np.asarray(i["Wso3"], f32) + np.asarray(i["bso3"], f32)
    ctrl = np.zeros((len(rows), 3), f32)
    for k in range(NBRANCH):
        m = cmd == k
        if not m.any():
            continue
        h = np.maximum(emb[m] @ np.asarray(i["Wb1"], f32)[k]
                       + np.asarray(i["bb1"], f32)[k], 0)
        h = np.maximum(h @ np.asarray(i["Wb2"], f32)[k]
                       + np.asarray(i["bb2"], f32)[k], 0)
        z = h @ np.asarray(i["Wb3"], f32)[k] + np.asarray(i["bb3"], f32)[k]
        ctrl[m] = 1.0 / (1.0 + np.exp(-z))
    return ctrl, spd.astype(f32)


# --------------------------------------------------------------------------
# Entry point
# --------------------------------------------------------------------------

LAST_RESULTS = None  # BassKernelResults of the most recent device run


def kernel(embedding, speed, command, **weights):
    global LAST_RESULTS
    inputs = dict(weights)
    inputs.update(embedding=embedding, speed=speed, command=command)

    embedding = np.asarray(embedding, dtype=np.float32)
    speed = np.asarray(speed, dtype=np.float32)
    command_np = np.asarray(command)

    if embedding.shape != (B, D_EMB):
        # Unexpected problem size: fall back to exact host computation.
        ctrl, spd = _np_reference(inputs, np.arange(embedding.shape[0]))
        return ctrl, spd

    cmd = command_np.astype(np.int64) - 1
    idx, valid, spill = _route(cmd)

    w = _prep_weights(inputs)

    in_maps = []
    for c in range(NCORES):
        rows = idx[c]
        xt = np.empty((D_EMB + 1, BPC), dtype=np.float16)
        xt[:D_EMB] = embedding[rows].T
        xt[D_EMB] = speed[rows, 0]
        m = {"xt": xt}
        m.update(w)
        in_maps.append(m)

    from concourse.bass_utils import run_bass_kernel_spmd

    nc = _build_nc()
    res = run_bass_kernel_spmd(
        nc, in_maps, core_ids=list(range(NCORES)),
        trace=bool(int(os.environ.get("KERNEL_TRACE", "0"))),
    )
    LAST_RESULTS = res

    control = np.zeros((B, 3), dtype=np.float32)
    speed_pred = np.zeros((B, 1), dtype=np.float32)
    for c in range(NCORES):
        o4 = np.asarray(res.results[c]["out4"])
        v = valid[c]
        rows = idx[c][v]
        control[rows] = o4[0:3, v].T
        speed_pred[rows, 0] = o4[3, v]

    if len(spill):
        ctrl_sp, spd_sp = _np_reference(inputs, spill)
        control[spill] = ctrl_sp
        speed_pred[spill] = spd_sp

    return control, speed_pred
